# revision 20
# baseline (speedup 1.0000x reference)
"""Tsit5 single-step neural-ODE kernel for TRN2 (8 NeuronCores, data parallel).

Network (per RHS eval, 6 evals per Tsit5 step):
    h  = concat(y_i, actions)          [80]
    h1 = silu(h @ W1 + b1)             [256]
    h2 = silu(h1 @ W2 + b2)            [256]
    k  = h2 @ W3 + b3                  [64]

Layout: feature-major activations [feat, batch] so the matmul contraction
dim sits on SBUF partitions.  Batch processed in pairs of 1024 columns
(elementwise at [*, 1024]; matmuls slice 512 columns = one PSUM bank).

dtypes: matmul operands and the Runge-Kutta k tiles are bf16 (full PE
rate, pipelined weight loads, DVE 2x mode); the y0 stash, the final y1
accumulation and the output path stay fp32 so the result keeps fp32-level
precision (y1 = y0 + small increment).
"""

import numpy as np

import concourse.bass as bass
import concourse.mybir as mybir
from concourse import bacc
from concourse.masks import make_identity
from concourse.tile import TileContext

F32 = mybir.dt.float32
BF16 = mybir.dt.bfloat16

OBS = 64
ACT = 16
DIN = 80
HID = 256
BATCH = 131072
NCORES = 8
BC = BATCH // NCORES
DT = 0.05

A_COEF = [
    [],
    [0.161],
    [-0.008480655492356989, 0.335480655492357],
    [2.8971530571054935, -6.359448489975075, 4.3622954328695815],
    [5.325864828439257, -11.748883564062828, 7.4955393428898365,
     -0.09249506636175525],
    [5.86145544294642, -12.92096931784711, 8.159367898576159,
     -0.071584973281401, -0.028269050394068383],
]
B_COEF = [0.09646076681806523, 0.01, 0.4798896504144996, 1.379008574103742,
          -3.290069515436081, 2.324710524099774]

PAIR = 1024
CHUNK = 512


def build(bc=BC, act_dt=BF16, sim_safe_silu=False):
    nc = bacc.Bacc("TRN2", target_bir_lowering=False, debug=False)

    obs_d = nc.dram_tensor("initial_obs", [bc, OBS], F32, kind="ExternalInput").ap()
    act_d = nc.dram_tensor("actions", [bc, ACT], F32, kind="ExternalInput").ap()
    w1_d = nc.dram_tensor("W1", [DIN, HID], F32, kind="ExternalInput").ap()
    b1_d = nc.dram_tensor("b1", [HID], F32, kind="ExternalInput").ap()
    w2_d = nc.dram_tensor("W2", [HID, HID], F32, kind="ExternalInput").ap()
    b2_d = nc.dram_tensor("b2", [HID], F32, kind="ExternalInput").ap()
    w3_d = nc.dram_tensor("W3", [HID, OBS], F32, kind="ExternalInput").ap()
    b3_d = nc.dram_tensor("b3", [OBS], F32, kind="ExternalInput").ap()
    out_d = nc.dram_tensor("out", [bc, OBS], F32, kind="ExternalOutput").ap()

    b1_c = b1_d.rearrange("(p o) -> p o", o=1)
    b2_c = b2_d.rearrange("(p o) -> p o", o=1)
    b3_c = b3_d.rearrange("(p o) -> p o", o=1)

    npairs = bc // PAIR
    assert bc % PAIR == 0
    nj = PAIR // 128  # 128-row blocks per pair

    silu = mybir.ActivationFunctionType.Silu
    sigmoid = mybir.ActivationFunctionType.Sigmoid
    add_op = mybir.AluOpType.add
    mult_op = mybir.AluOpType.mult

    def emit_silu(out, ph, bias, pool, tag):
        """out = silu(ph + bias). CoreSim lacks Silu; sim mode decomposes."""
        if not sim_safe_silu:
            nc.scalar.activation(out, ph, silu, bias=bias)
        else:
            z = pool.tile(list(out.shape), F32, name=f"z_{tag}",
                          tag=f"z_{tag}", bufs=2)
            nc.vector.tensor_scalar(z, ph, bias, None, op0=add_op)
            nc.scalar.activation(out, z, sigmoid)
            nc.vector.tensor_tensor(out, out, z, op=mult_op)

    with TileContext(nc) as tc:
        with (
            tc.tile_pool(name="const", bufs=1) as const,
            tc.tile_pool(name="io", bufs=1) as io,
            tc.tile_pool(name="acts", bufs=1) as acts,
            tc.tile_pool(name="ptr", bufs=1, space="PSUM") as ptr_pool,
            tc.tile_pool(name="ph", bufs=1, space="PSUM") as ph_pool,
        ):
            # ---- constants: identity + weights (cast to bf16) + biases ----
            ident = const.tile([128, 128], F32)
            make_identity(nc, ident)

            w1 = []
            for n in range(2):
                w1s = io.tile([DIN, 128], F32, name=f"w1s_{n}", tag="wstage",
                              bufs=8)
                nc.sync.dma_start(w1s, w1_d[:, n * 128:(n + 1) * 128])
                w1n = const.tile([DIN, 128], act_dt, name=f"w1_{n}")
                nc.vector.tensor_copy(w1n, w1s)
                w1.append(w1n)
            w2 = {}
            for k in range(2):
                for n in range(2):
                    w2s = io.tile([128, 128], F32, name=f"w2s_{k}{n}",
                                  tag="wstage", bufs=8)
                    nc.sync.dma_start(
                        w2s, w2_d[k * 128:(k + 1) * 128, n * 128:(n + 1) * 128])
                    w2kn = const.tile([128, 128], act_dt, name=f"w2_{k}{n}")
                    nc.vector.tensor_copy(w2kn, w2s)
                    w2[(k, n)] = w2kn
            w3 = []
            for k in range(2):
                w3s = io.tile([128, OBS], F32, name=f"w3s_{k}", tag="wstage",
                              bufs=8)
                nc.sync.dma_start(w3s, w3_d[k * 128:(k + 1) * 128, :])
                w3k = const.tile([128, OBS], act_dt, name=f"w3_{k}")
                nc.vector.tensor_copy(w3k, w3s)
                w3.append(w3k)

            b1t = []
            b2t = []
            for n in range(2):
                b1n = const.tile([128, 1], F32, name=f"b1_{n}")
                nc.sync.dma_start(b1n, b1_c[n * 128:(n + 1) * 128, :])
                b1t.append(b1n)
                b2n = const.tile([128, 1], F32, name=f"b2_{n}")
                nc.sync.dma_start(b2n, b2_c[n * 128:(n + 1) * 128, :])
                b2t.append(b2n)
            b3t = const.tile([OBS, 1], F32)
            nc.sync.dma_start(b3t, b3_c)
            db3 = const.tile([OBS, 1], F32)
            nc.vector.tensor_scalar(db3, b3t, DT, None, op0=mult_op)

            # ---- per-pair pipeline (two pairs interleaved) ----
            def emit_load(p):
                r0 = p * PAIR
                st = {}
                st["yt"] = acts.tile([DIN, PAIR], act_dt, name="yt", tag="yt",
                                     bufs=3)
                st["y0b"] = acts.tile([OBS, PAIR], act_dt, name="y0b",
                                      tag="y0b", bufs=3)
                st["y0t"] = acts.tile([OBS, PAIR], F32, name="y0t", tag="y0t",
                                      bufs=3)
                yt, y0b, y0t = st["yt"], st["y0b"], st["y0t"]
                for g in range(nj // 4):
                    ptro_ = ptr_pool.tile([OBS, 512], F32, name="ptro_",
                                          tag="tr", bufs=2)
                    ptra_ = ptr_pool.tile([ACT, 512], F32, name="ptra_",
                                          tag="tr", bufs=2)
                    for jj in range(4):
                        j = g * 4 + jj
                        ito = io.tile([128, OBS], F32, name="ito", tag="ito",
                                      bufs=16)
                        nc.sync.dma_start(
                            ito, obs_d[r0 + j * 128:r0 + (j + 1) * 128, :])
                        nc.tensor.transpose(
                            ptro_[:, jj * 128:(jj + 1) * 128], ito, ident)
                        ita = io.tile([128, ACT], F32, name="ita", tag="ita",
                                      bufs=16)
                        nc.sync.dma_start(
                            ita, act_d[r0 + j * 128:r0 + (j + 1) * 128, :])
                        nc.tensor.transpose(
                            ptra_[:, jj * 128:(jj + 1) * 128], ita, ident)
                    cs = slice(g * 512, (g + 1) * 512)
                    nc.vector.tensor_copy(yt[0:OBS, cs], ptro_)
                    nc.vector.tensor_copy(y0t[:, cs], ptro_)
                    nc.vector.tensor_copy(yt[OBS:DIN, cs], ptra_)
                nc.vector.tensor_copy(y0b, yt[0:OBS, :])
                st["accs"] = [None] * 6
                st["y1a"] = None
                return st

            def emit_mlp_l1(st):
                yt = st["yt"]
                h1t = []
                for n in range(2):
                    ph = ph_pool.tile([128, PAIR], F32, name=f"ph1_{n}",
                                      tag="ph", bufs=3)
                    for c in range(2):
                        nc.tensor.matmul(
                            ph[:, c * CHUNK:(c + 1) * CHUNK],
                            w1[n],
                            yt[:, c * CHUNK:(c + 1) * CHUNK],
                            start=True, stop=True)
                    h1n = acts.tile([128, PAIR], act_dt, name=f"h1t_{n}",
                                    tag="h1t", bufs=6)
                    emit_silu(h1n, ph, b1t[n], acts, "h1")
                    h1t.append(h1n)
                return h1t

            def emit_mlp_l2(st, h1t):
                ph2 = []
                for n in range(2):
                    ph2.append(ph_pool.tile(
                        [128, PAIR], F32, name=f"ph2_{n}", tag="ph", bufs=3))
                for k in range(2):
                    for n in range(2):
                        for c in range(2):
                            nc.tensor.matmul(
                                ph2[n][:, c * CHUNK:(c + 1) * CHUNK],
                                w2[(k, n)],
                                h1t[k][:, c * CHUNK:(c + 1) * CHUNK],
                                start=(k == 0), stop=(k == 1))
                h2t = []
                for n in range(2):
                    h2n = acts.tile([128, PAIR], act_dt, name=f"h2t_{n}",
                                    tag="h2t", bufs=6)
                    emit_silu(h2n, ph2[n], b2t[n], acts, "h2")
                    h2t.append(h2n)
                return h2t

            def emit_mlp_l3(st, h2t, i):
                yt = st["yt"]
                pk = ph_pool.tile([OBS, PAIR], F32, name="pk", tag="ph",
                                  bufs=3)
                for k in range(2):
                    for c in range(2):
                        nc.tensor.matmul(
                            pk[:, c * CHUNK:(c + 1) * CHUNK],
                            w3[k],
                            h2t[k][:, c * CHUNK:(c + 1) * CHUNK],
                            start=(k == 0), stop=(k == 1))
                kt = acts.tile([OBS, PAIR], act_dt, name="kt", tag="kt",
                               bufs=14)
                if sim_safe_silu:
                    nc.vector.tensor_scalar(kt, pk, b3t, DT,
                                            op0=add_op, op1=mult_op)
                elif i % 2 == 0:
                    nc.scalar.activation(
                        kt, pk, mybir.ActivationFunctionType.Identity,
                        bias=db3, scale=DT)
                else:
                    nc.vector.tensor_scalar(kt, pk, b3t, DT,
                                            op0=add_op, op1=mult_op)

                # critical fold first: y_{i+1} straight into yt's obs rows
                if i + 1 < 6:
                    src1 = st["y0b"] if st["accs"][i + 1] is None \
                        else st["accs"][i + 1]
                    nc.vector.scalar_tensor_tensor(
                        yt[0:OBS, :], kt, float(A_COEF[i + 1][i]), src1,
                        op0=mult_op, op1=add_op)
                # remaining folds: tensor_scalar (4x) + tensor_tensor (2x)
                # beats the fused scalar_tensor_tensor (stuck at 1x)
                for ii in range(i + 2, 6):
                    src1 = st["y0b"] if st["accs"][ii] is None \
                        else st["accs"][ii]
                    tmp = acts.tile([OBS, PAIR], act_dt, name="ftmp",
                                    tag="ftmp", bufs=4)
                    nc.vector.tensor_scalar(tmp, kt, float(A_COEF[ii][i]),
                                            None, op0=mult_op)
                    acc = acts.tile([OBS, PAIR], act_dt, name=f"acc_{ii}",
                                    tag=f"acc{ii}", bufs=4)
                    nc.vector.tensor_tensor(acc, tmp, src1, op=add_op)
                    st["accs"][ii] = acc
                src1 = st["y0t"] if st["y1a"] is None else st["y1a"]
                y1n = acts.tile([OBS, PAIR], F32, name="y1a", tag="y1a",
                                bufs=4)
                nc.vector.scalar_tensor_tensor(
                    y1n, kt, float(B_COEF[i]), src1,
                    op0=mult_op, op1=add_op)
                st["y1a"] = y1n

            def emit_out(p, st):
                r0 = p * PAIR
                y1t = st["y1a"]
                for g in range(nj // 4):
                    pto = ptr_pool.tile([128, 256], F32, name="pto",
                                        tag="tr", bufs=2)
                    for jj in range(4):
                        j = g * 4 + jj
                        nc.tensor.transpose(
                            pto[:, jj * OBS:(jj + 1) * OBS],
                            y1t[:, j * 128:(j + 1) * 128],
                            ident[0:OBS, 0:OBS])
                    ot = io.tile([128, 256], F32, name="out_t", tag="out_t",
                                 bufs=8)
                    nc.vector.tensor_copy(ot, pto)
                    for jj in range(4):
                        j = g * 4 + jj
                        nc.sync.dma_start(
                            out_d[r0 + j * 128:r0 + (j + 1) * 128, :],
                            ot[:, jj * OBS:(jj + 1) * OBS])

            assert npairs % 2 == 0
            # software pipeline: next group's loads are emitted mid-way
            # through the current group's evals so the transposes/DMAs fill
            # engine slack instead of serializing at the boundary
            states = None
            for pp in range(0, npairs, 2):
                if states is None:
                    states = (emit_load(pp), emit_load(pp + 1))
                stA, stB = states
                nxt = None
                for i in range(6):
                    h1A = emit_mlp_l1(stA)
                    h1B = emit_mlp_l1(stB)
                    h2A = emit_mlp_l2(stA, h1A)
                    h2B = emit_mlp_l2(stB, h1B)
                    emit_mlp_l3(stA, h2A, i)
                    emit_mlp_l3(stB, h2B, i)
                    if i == 2 and pp + 2 < npairs:
                        nxt = (emit_load(pp + 2), emit_load(pp + 3))
                emit_out(pp, stA)
                emit_out(pp + 1, stB)
                states = nxt

    nc.finalize()
    return nc


_NC_CACHE = {}


def _get_nc(bc=BC):
    if bc not in _NC_CACHE:
        _NC_CACHE[bc] = build(bc)
    return _NC_CACHE[bc]


def run(inputs, trace=False, **kw):
    """Shard inputs across 8 cores, run, gather. Returns (out, results)."""
    from concourse.bass_utils import run_bass_kernel_spmd

    obs = np.ascontiguousarray(np.asarray(inputs["initial_obs"], np.float32))
    act = np.ascontiguousarray(np.asarray(inputs["actions"], np.float32))
    shared = {
        k: np.ascontiguousarray(np.asarray(inputs[k], np.float32))
        for k in ["W1", "b1", "W2", "b2", "W3", "b3"]
    }
    bc = obs.shape[0] // NCORES
    nc = _get_nc(bc)
    in_maps = []
    for c in range(NCORES):
        sl = slice(c * bc, (c + 1) * bc)
        in_maps.append({
            "initial_obs": np.ascontiguousarray(obs[sl]),
            "actions": np.ascontiguousarray(act[sl]),
            **shared,
        })
    res = run_bass_kernel_spmd(nc, in_maps, core_ids=list(range(NCORES)),
                               trace=trace, **kw)
    out = np.concatenate([r["out"] for r in res.results], axis=0)
    return out, res


def kernel(**inputs):
    out, _ = run(inputs)
    return out


# revision 21
# speedup vs baseline: 1.2663x; 1.2663x over previous
"""Tsit5 single-step neural-ODE kernel for TRN2 (8 NeuronCores, data parallel).

Network (per RHS eval, 6 evals per Tsit5 step):
    h  = concat(y_i, actions)          [80]
    h1 = silu(h @ W1 + b1)             [256]
    h2 = silu(h1 @ W2 + b2)            [256]
    k  = h2 @ W3 + b3                  [64]

Layout: feature-major activations [feat, batch] so the matmul contraction
dim sits on SBUF partitions.  Batch processed in pairs of 1024 columns
(elementwise at [*, 1024]; matmuls slice 512 columns = one PSUM bank).

dtypes: matmul operands and the Runge-Kutta k tiles are bf16 (full PE
rate, pipelined weight loads, DVE 2x mode); the y0 stash, the final y1
accumulation and the output path stay fp32 so the result keeps fp32-level
precision (y1 = y0 + small increment).
"""

import numpy as np

import concourse.bass as bass
import concourse.mybir as mybir
from concourse import bacc
from concourse.masks import make_identity
from concourse.tile import TileContext

F32 = mybir.dt.float32
BF16 = mybir.dt.bfloat16

OBS = 64
ACT = 16
DIN = 80
HID = 256
BATCH = 131072
NCORES = 8
BC = BATCH // NCORES
DT = 0.05

A_COEF = [
    [],
    [0.161],
    [-0.008480655492356989, 0.335480655492357],
    [2.8971530571054935, -6.359448489975075, 4.3622954328695815],
    [5.325864828439257, -11.748883564062828, 7.4955393428898365,
     -0.09249506636175525],
    [5.86145544294642, -12.92096931784711, 8.159367898576159,
     -0.071584973281401, -0.028269050394068383],
]
B_COEF = [0.09646076681806523, 0.01, 0.4798896504144996, 1.379008574103742,
          -3.290069515436081, 2.324710524099774]

PAIR = 1024
CHUNK = 512


def build(bc=BC, act_dt=BF16, sim_safe_silu=False):
    nc = bacc.Bacc("TRN2", target_bir_lowering=False, debug=False)

    obs_d = nc.dram_tensor("initial_obs", [bc, OBS], F32, kind="ExternalInput").ap()
    act_d = nc.dram_tensor("actions", [bc, ACT], F32, kind="ExternalInput").ap()
    w1_d = nc.dram_tensor("W1", [DIN, HID], F32, kind="ExternalInput").ap()
    b1_d = nc.dram_tensor("b1", [HID], F32, kind="ExternalInput").ap()
    w2_d = nc.dram_tensor("W2", [HID, HID], F32, kind="ExternalInput").ap()
    b2_d = nc.dram_tensor("b2", [HID], F32, kind="ExternalInput").ap()
    w3_d = nc.dram_tensor("W3", [HID, OBS], F32, kind="ExternalInput").ap()
    b3_d = nc.dram_tensor("b3", [OBS], F32, kind="ExternalInput").ap()
    out_d = nc.dram_tensor("out", [bc, OBS], F32, kind="ExternalOutput").ap()

    b1_c = b1_d.rearrange("(p o) -> p o", o=1)
    b2_c = b2_d.rearrange("(p o) -> p o", o=1)
    b3_c = b3_d.rearrange("(p o) -> p o", o=1)

    npairs = bc // PAIR
    assert bc % PAIR == 0
    nj = PAIR // 128  # 128-row blocks per pair

    silu = mybir.ActivationFunctionType.Silu
    sigmoid = mybir.ActivationFunctionType.Sigmoid
    add_op = mybir.AluOpType.add
    mult_op = mybir.AluOpType.mult

    def emit_silu(out, ph, bias, pool, tag):
        """out = silu(ph + bias). CoreSim lacks Silu; sim mode decomposes."""
        if not sim_safe_silu:
            nc.scalar.activation(out, ph, silu, bias=bias)
        else:
            z = pool.tile(list(out.shape), F32, name=f"z_{tag}",
                          tag=f"z_{tag}", bufs=2)
            nc.vector.tensor_scalar(z, ph, bias, None, op0=add_op)
            nc.scalar.activation(out, z, sigmoid)
            nc.vector.tensor_tensor(out, out, z, op=mult_op)

    with TileContext(nc) as tc:
        with (
            tc.tile_pool(name="const", bufs=1) as const,
            tc.tile_pool(name="io", bufs=1) as io,
            tc.tile_pool(name="acts", bufs=1) as acts,
            tc.tile_pool(name="ptr", bufs=1, space="PSUM") as ptr_pool,
            tc.tile_pool(name="ph", bufs=1, space="PSUM") as ph_pool,
        ):
            # ---- constants: identity + weights (cast to bf16) + biases ----
            ident = const.tile([128, 128], F32)
            make_identity(nc, ident)

            w1 = []
            for n in range(2):
                w1s = io.tile([DIN, 128], F32, name=f"w1s_{n}", tag="wstage",
                              bufs=8)
                nc.sync.dma_start(w1s, w1_d[:, n * 128:(n + 1) * 128])
                w1n = const.tile([DIN, 128], act_dt, name=f"w1_{n}")
                nc.vector.tensor_copy(w1n, w1s)
                w1.append(w1n)
            w2 = {}
            for k in range(2):
                for n in range(2):
                    w2s = io.tile([128, 128], F32, name=f"w2s_{k}{n}",
                                  tag="wstage", bufs=8)
                    nc.sync.dma_start(
                        w2s, w2_d[k * 128:(k + 1) * 128, n * 128:(n + 1) * 128])
                    w2kn = const.tile([128, 128], act_dt, name=f"w2_{k}{n}")
                    nc.vector.tensor_copy(w2kn, w2s)
                    w2[(k, n)] = w2kn
            w3 = []
            for k in range(2):
                w3s = io.tile([128, OBS], F32, name=f"w3s_{k}", tag="wstage",
                              bufs=8)
                nc.sync.dma_start(w3s, w3_d[k * 128:(k + 1) * 128, :])
                w3k = const.tile([128, OBS], act_dt, name=f"w3_{k}")
                nc.vector.tensor_copy(w3k, w3s)
                w3.append(w3k)

            b1t = []
            b2t = []
            for n in range(2):
                b1n = const.tile([128, 1], F32, name=f"b1_{n}")
                nc.sync.dma_start(b1n, b1_c[n * 128:(n + 1) * 128, :])
                b1t.append(b1n)
                b2n = const.tile([128, 1], F32, name=f"b2_{n}")
                nc.sync.dma_start(b2n, b2_c[n * 128:(n + 1) * 128, :])
                b2t.append(b2n)
            b3t = const.tile([OBS, 1], F32)
            nc.sync.dma_start(b3t, b3_c)
            db3 = const.tile([OBS, 1], F32)
            nc.vector.tensor_scalar(db3, b3t, DT, None, op0=mult_op)

            # ---- per-pair pipeline (two pairs interleaved) ----
            def emit_load(p):
                r0 = p * PAIR
                st = {}
                st["yt"] = acts.tile([DIN, PAIR], act_dt, name="yt", tag="yt",
                                     bufs=3)
                st["y0b"] = acts.tile([OBS, PAIR], act_dt, name="y0b",
                                      tag="y0b", bufs=3)
                st["y0t"] = acts.tile([OBS, PAIR], F32, name="y0t", tag="y0t",
                                      bufs=3)
                yt, y0b, y0t = st["yt"], st["y0b"], st["y0t"]
                for g in range(nj // 4):
                    ptro_ = ptr_pool.tile([OBS, 512], F32, name="ptro_",
                                          tag="tr", bufs=2)
                    ptra_ = ptr_pool.tile([ACT, 512], F32, name="ptra_",
                                          tag="tr", bufs=2)
                    for jj in range(4):
                        j = g * 4 + jj
                        ito = io.tile([128, OBS], F32, name="ito", tag="ito",
                                      bufs=16)
                        nc.sync.dma_start(
                            ito, obs_d[r0 + j * 128:r0 + (j + 1) * 128, :])
                        nc.tensor.transpose(
                            ptro_[:, jj * 128:(jj + 1) * 128], ito, ident)
                        ita = io.tile([128, ACT], F32, name="ita", tag="ita",
                                      bufs=16)
                        nc.sync.dma_start(
                            ita, act_d[r0 + j * 128:r0 + (j + 1) * 128, :])
                        nc.tensor.transpose(
                            ptra_[:, jj * 128:(jj + 1) * 128], ita, ident)
                    cs = slice(g * 512, (g + 1) * 512)
                    nc.vector.tensor_copy(yt[0:OBS, cs], ptro_)
                    nc.vector.tensor_copy(y0t[:, cs], ptro_)
                    nc.vector.tensor_copy(yt[OBS:DIN, cs], ptra_)
                nc.vector.tensor_copy(y0b, yt[0:OBS, :])
                st["accs"] = [None] * 6
                st["y1a"] = None
                return st

            def emit_mlp_l1(st):
                yt = st["yt"]
                h1t = []
                for n in range(2):
                    ph = ph_pool.tile([128, PAIR], F32, name=f"ph1_{n}",
                                      tag="ph", bufs=3)
                    for c in range(2):
                        nc.tensor.matmul(
                            ph[:, c * CHUNK:(c + 1) * CHUNK],
                            w1[n],
                            yt[:, c * CHUNK:(c + 1) * CHUNK],
                            start=True, stop=True)
                    h1n = acts.tile([128, PAIR], act_dt, name=f"h1t_{n}",
                                    tag="h1t", bufs=6)
                    emit_silu(h1n, ph, b1t[n], acts, "h1")
                    h1t.append(h1n)
                return h1t

            def emit_mlp_l2(st, h1t):
                ph2 = []
                for n in range(2):
                    ph2.append(ph_pool.tile(
                        [128, PAIR], F32, name=f"ph2_{n}", tag="ph", bufs=3))
                for k in range(2):
                    for n in range(2):
                        for c in range(2):
                            nc.tensor.matmul(
                                ph2[n][:, c * CHUNK:(c + 1) * CHUNK],
                                w2[(k, n)],
                                h1t[k][:, c * CHUNK:(c + 1) * CHUNK],
                                start=(k == 0), stop=(k == 1))
                h2t = []
                for n in range(2):
                    h2n = acts.tile([128, PAIR], act_dt, name=f"h2t_{n}",
                                    tag="h2t", bufs=6)
                    emit_silu(h2n, ph2[n], b2t[n], acts, "h2")
                    h2t.append(h2n)
                return h2t

            def emit_mlp_l3(st, h2t, i):
                yt = st["yt"]
                pk = ph_pool.tile([OBS, PAIR], F32, name="pk", tag="ph",
                                  bufs=3)
                for k in range(2):
                    for c in range(2):
                        nc.tensor.matmul(
                            pk[:, c * CHUNK:(c + 1) * CHUNK],
                            w3[k],
                            h2t[k][:, c * CHUNK:(c + 1) * CHUNK],
                            start=(k == 0), stop=(k == 1))
                kt = acts.tile([OBS, PAIR], act_dt, name="kt", tag="kt",
                               bufs=14)
                if sim_safe_silu:
                    nc.vector.tensor_scalar(kt, pk, b3t, DT,
                                            op0=add_op, op1=mult_op)
                elif i % 2 == 0:
                    nc.scalar.activation(
                        kt, pk, mybir.ActivationFunctionType.Identity,
                        bias=db3, scale=DT)
                else:
                    nc.vector.tensor_scalar(kt, pk, b3t, DT,
                                            op0=add_op, op1=mult_op)

                # critical fold first: y_{i+1} straight into yt's obs rows
                if i + 1 < 6:
                    src1 = st["y0b"] if st["accs"][i + 1] is None \
                        else st["accs"][i + 1]
                    nc.vector.scalar_tensor_tensor(
                        yt[0:OBS, :], kt, float(A_COEF[i + 1][i]), src1,
                        op0=mult_op, op1=add_op)
                # remaining folds: tensor_scalar (4x) + tensor_tensor (2x)
                # beats the fused scalar_tensor_tensor (stuck at 1x)
                for ii in range(i + 2, 6):
                    src1 = st["y0b"] if st["accs"][ii] is None \
                        else st["accs"][ii]
                    tmp = acts.tile([OBS, PAIR], act_dt, name="ftmp",
                                    tag="ftmp", bufs=4)
                    nc.vector.tensor_scalar(tmp, kt, float(A_COEF[ii][i]),
                                            None, op0=mult_op)
                    acc = acts.tile([OBS, PAIR], act_dt, name=f"acc_{ii}",
                                    tag=f"acc{ii}", bufs=4)
                    nc.vector.tensor_tensor(acc, tmp, src1, op=add_op)
                    st["accs"][ii] = acc
                src1 = st["y0t"] if st["y1a"] is None else st["y1a"]
                y1n = acts.tile([OBS, PAIR], F32, name="y1a", tag="y1a",
                                bufs=4)
                nc.vector.scalar_tensor_tensor(
                    y1n, kt, float(B_COEF[i]), src1,
                    op0=mult_op, op1=add_op)
                st["y1a"] = y1n

            def emit_out(p, st):
                r0 = p * PAIR
                y1t = st["y1a"]
                for g in range(nj // 4):
                    pto = ptr_pool.tile([128, 256], F32, name="pto",
                                        tag="tr", bufs=2)
                    for jj in range(4):
                        j = g * 4 + jj
                        nc.tensor.transpose(
                            pto[:, jj * OBS:(jj + 1) * OBS],
                            y1t[:, j * 128:(j + 1) * 128],
                            ident[0:OBS, 0:OBS])
                    ot = io.tile([128, 256], F32, name="out_t", tag="out_t",
                                 bufs=8)
                    nc.vector.tensor_copy(ot, pto)
                    for jj in range(4):
                        j = g * 4 + jj
                        nc.sync.dma_start(
                            out_d[r0 + j * 128:r0 + (j + 1) * 128, :],
                            ot[:, jj * OBS:(jj + 1) * OBS])

            assert npairs % 2 == 0
            # software pipeline: next group's loads are emitted mid-way
            # through the current group's evals so the transposes/DMAs fill
            # engine slack instead of serializing at the boundary
            states = None
            for pp in range(0, npairs, 2):
                if states is None:
                    states = (emit_load(pp), emit_load(pp + 1))
                stA, stB = states
                nxt = None
                for i in range(6):
                    emit_mlp_l3(stA, emit_mlp_l2(stA, emit_mlp_l1(stA)), i)
                    emit_mlp_l3(stB, emit_mlp_l2(stB, emit_mlp_l1(stB)), i)
                    if i == 2 and pp + 2 < npairs:
                        nxt = (emit_load(pp + 2), emit_load(pp + 3))
                emit_out(pp, stA)
                emit_out(pp + 1, stB)
                states = nxt

    nc.finalize()
    return nc


_NC_CACHE = {}


def _get_nc(bc=BC):
    if bc not in _NC_CACHE:
        _NC_CACHE[bc] = build(bc)
    return _NC_CACHE[bc]


def run(inputs, trace=False, **kw):
    """Shard inputs across 8 cores, run, gather. Returns (out, results)."""
    from concourse.bass_utils import run_bass_kernel_spmd

    obs = np.ascontiguousarray(np.asarray(inputs["initial_obs"], np.float32))
    act = np.ascontiguousarray(np.asarray(inputs["actions"], np.float32))
    shared = {
        k: np.ascontiguousarray(np.asarray(inputs[k], np.float32))
        for k in ["W1", "b1", "W2", "b2", "W3", "b3"]
    }
    bc = obs.shape[0] // NCORES
    nc = _get_nc(bc)
    in_maps = []
    for c in range(NCORES):
        sl = slice(c * bc, (c + 1) * bc)
        in_maps.append({
            "initial_obs": np.ascontiguousarray(obs[sl]),
            "actions": np.ascontiguousarray(act[sl]),
            **shared,
        })
    res = run_bass_kernel_spmd(nc, in_maps, core_ids=list(range(NCORES)),
                               trace=trace, **kw)
    out = np.concatenate([r["out"] for r in res.results], axis=0)
    return out, res


def kernel(**inputs):
    out, _ = run(inputs)
    return out
